# revision 20
# baseline (speedup 1.0000x reference)
"""Trainium2 Bass kernel for shifted sparse attention (nn_Attention_74672301408506).

Math (reference):
    q = x @ W.T ; k = x_key @ W.T ; att = softmax(q k^T)
    out[i, v] = sum_s w_s * sum_j att[i-2s, j] * x_value[j+2s, v]

Device algorithm (8 cores, query rows sharded, 8 halo rows recomputed):
    per core, with local query window rows [r0-8, r0+512):
      qT[h, i], kT[h, j]   (float32r matmuls)
      S^T[j, i] = kT^T q   (scores transposed: keys on partitions)
      E = exp(S - 110)     (bf16; softmax shift-invariant, 110 avoids overflow)
      Ru[i, 0:320|320] = E^T @ [V_0|V_1|V_2|V_3|ones]   (ones col = softmax denom)
      R = Ru[:, :320] * mask / Ru[:, 320]
      out[p, v] = sum_s w_s R[p + 8 - 2s, 80s + v]  (banded-matrix matmuls)
"""

import os
import sys
import types

import numpy as np
import ml_dtypes

T = 4096
Q = 256
H = 256
NV = 80
NS = 4
STEP = 2
NCORES = 8
M = T // NCORES            # 512 rows per core
HALO = 8                   # = (NS-1)*STEP + slack, multiple of 2
MH = M + HALO              # 520 i-window per core
CSUB = 110.0               # global score shift before exp
P = 128
NJ = T // P                # 32 key tiles
NF = Q // P                # 2 feature tiles
NH = H // P                # 2 hidden tiles
NMAIN = M // P             # 4 main i-chunks of 128


def _install_axon_ntff_hook():
    """bass_utils' trace path imports antenv.axon_hooks, which the agent image
    lacks; shim it and register the ctypes-based NTFF profiler hook."""
    if "antenv.axon_hooks" in sys.modules:
        return
    try:
        import antenv
    except ImportError:
        return
    mod = types.ModuleType("antenv.axon_hooks")
    mod._hook = None
    mod.set_axon_ntff_profile_hook = lambda h: setattr(mod, "_hook", h)
    mod.get_axon_ntff_profile_hook = lambda: mod._hook
    sys.modules["antenv.axon_hooks"] = mod
    antenv.axon_hooks = mod
    try:
        from trn_agent_boot import trn_boot

        so_path = "/opt/axon/libaxon_pjrt.so"
        if os.path.exists(so_path):
            mod.set_axon_ntff_profile_hook(trn_boot._ntff_profile_via_ctypes(so_path))
    except Exception:
        pass


_NC_CACHE = {}
LAST_RESULT = None


def _build_nc():
    import concourse.mybir as mybir
    import concourse.tile as tile
    from concourse import bacc

    f32 = mybir.dt.float32
    f32r = mybir.dt.float32r
    bf16 = mybir.dt.bfloat16
    Exp = mybir.ActivationFunctionType.Exp

    nc = bacc.Bacc(None, target_bir_lowering=False)

    xT_d = nc.dram_tensor("xT", [Q, MH], f32r, kind="ExternalInput")
    xkT_d = nc.dram_tensor("xkT", [Q, T], f32r, kind="ExternalInput")
    wT_d = nc.dram_tensor("wT", [Q, H], f32r, kind="ExternalInput")
    vc_d = nc.dram_tensor("vcomb", [NJ, P, NS * NV + 1], bf16, kind="ExternalInput")
    aux_d = nc.dram_tensor("aux", [P, 2 * NS * P], bf16, kind="ExternalInput")
    masks_d = nc.dram_tensor("masks", [P, 2], f32, kind="ExternalInput")
    out_d = nc.dram_tensor("out", [NMAIN, P, NV], f32, kind="ExternalOutput")

    with tile.TileContext(nc) as tc:
        with (
            tc.tile_pool(name="consts", bufs=1) as consts,
            tc.tile_pool(name="io", bufs=3) as io,
            tc.tile_pool(name="store", bufs=1) as store,
            tc.tile_pool(name="small", bufs=6) as small,
            tc.tile_pool(name="psA", bufs=2, space="PSUM") as psA,
            tc.tile_pool(name="psB", bufs=1, space="PSUM") as psB,
            tc.tile_pool(name="psR", bufs=4, space="PSUM") as psR,
            tc.tile_pool(name="psR8", bufs=1, space="PSUM") as psR8,
        ):
            # ---- constants / small inputs (gpsimd queue; sync queue is for
            # the latency-critical xk stream) ----
            aux = consts.tile([P, 2 * NS * P], bf16, name="aux")
            nc.gpsimd.dma_start(out=aux, in_=aux_d[:, :])
            sh1 = aux[:, 0 : NS * P]
            sh2 = aux[0:HALO, NS * P : 2 * NS * P]
            masks = consts.tile([P, 2], f32, name="masks")
            nc.gpsimd.dma_start(out=masks, in_=masks_d[:, :])
            rmask = masks[:, 0:1]
            rminv = masks[:, 1:2]
            bias_t = consts.tile([P, 1], f32, name="bias_t")
            nc.vector.memset(bias_t, -CSUB)

            wt = []
            for f in range(NF):
                t = consts.tile([P, H], f32r, name=f"wt{f}", tag=f"wt{f}")
                nc.sync.dma_start(out=t, in_=wT_d[P * f : P * (f + 1), :])
                wt.append(t)
            xt = []
            for f in range(NF):
                t = consts.tile([P, MH], f32r, name=f"xt{f}", tag=f"xt{f}")
                nc.sync.dma_start(out=t, in_=xT_d[P * f : P * (f + 1), :])
                xt.append(t)

            # value tiles (bf16, includes ones column): ONE strided DMA
            # (DMA-issue instructions cost ~650ns each on the queue engine)
            NVC = NS * NV + 1
            vcall = store.tile([P, NJ, NVC], bf16, name="vcall", tag="vc")
            nc.gpsimd.dma_start(
                out=vcall, in_=vc_d.rearrange("j p c -> p j c")
            )
            vc = [vcall[:, j, :] for j in range(NJ)]

            # ---- phase A: qT, kT ----
            qt = []
            for h in range(NH):
                ps = psA.tile([P, M], f32, name=f"qps{h}", tag="ps")
                for f in range(NF):
                    nc.tensor.matmul(
                        ps,
                        wt[f][:, P * h : P * (h + 1)],
                        xt[f][:, 0:M],
                        start=(f == 0),
                        stop=(f == NF - 1),
                    )
                ps8 = psB.tile([P, HALO], f32, name=f"qps8{h}", tag="ps8")
                for f in range(NF):
                    nc.tensor.matmul(
                        ps8,
                        wt[f][:, P * h : P * (h + 1)],
                        xt[f][:, M:MH],
                        start=(f == 0),
                        stop=(f == NF - 1),
                    )
                t = store.tile([P, MH], f32r, name=f"qt{h}", tag=f"qt{h}")
                if h == 0:
                    nc.vector.tensor_copy(t[:, 0:M], ps)
                else:
                    nc.scalar.copy(t[:, 0:M], ps)
                nc.vector.tensor_copy(t[:, M:MH], ps8)
                qt.append(t)

            kt = []
            for h in range(NH):
                kt.append(store.tile([P, T], f32r, name=f"kt{h}", tag=f"kt{h}"))
            NKC = 8  # 512-wide key chunks
            xkh = []
            for f in range(NF):
                halves = []
                for half in range(2):
                    t = io.tile(
                        [P, T // 2], f32r, name=f"xkh{f}_{half}", tag=f"xk{f}", bufs=2
                    )
                    nc.sync.dma_start(
                        out=t,
                        in_=xkT_d[
                            P * f : P * (f + 1), (T // 2) * half : (T // 2) * (half + 1)
                        ],
                    )
                    halves.append(t)
                xkh.append(halves)
            for jc in range(NKC):
                xkc = [
                    xkh[f][jc // 4][:, 512 * (jc % 4) : 512 * (jc % 4 + 1)]
                    for f in range(NF)
                ]
                for h in range(NH):
                    ps = psA.tile([P, 512], f32, name=f"kps{jc}_{h}", tag="ps")
                    for f in range(NF):
                        nc.tensor.matmul(
                            ps,
                            wt[f][:, P * h : P * (h + 1)],
                            xkc[f],
                            start=(f == 0),
                            stop=(f == NF - 1),
                        )
                    dst = kt[h][:, 512 * jc : 512 * (jc + 1)]
                    if (jc + h) % 2 == 0:
                        nc.vector.tensor_copy(dst, ps)
                    else:
                        nc.scalar.copy(dst, ps)

            # ---- phases B+C interleaved: scores^T -> exp -> Ru accumulation ----
            elist = []
            ru = []
            for c in range(NMAIN):
                ru.append(psR.tile([P, NS * NV + 1], f32, name=f"ru{c}", tag="ru"))
            ru8 = psR8.tile([HALO, NS * NV + 1], f32, name="ru8", tag="ru8")
            e8 = store.tile([P, 8 * HALO * 4], bf16, name="e8", tag="e8")
            # e8 groups: 4 groups of 8 j-tiles, each segment [64] wide
            ps8g = None
            for j in range(NJ):
                g, jg = divmod(j, 8)
                ps = psA.tile([P, M], f32, name=f"sps{j}", tag="ps")
                for h in range(NH):
                    nc.tensor.matmul(
                        ps,
                        kt[h][:, P * j : P * (j + 1)],
                        qt[h][:, 0:M],
                        start=(h == 0),
                        stop=(h == NH - 1),
                    )
                if jg == 0:
                    ps8g = psB.tile([P, 8 * HALO], f32, name=f"ps8g{g}", tag="ps8")
                for h in range(NH):
                    nc.tensor.matmul(
                        ps8g[:, HALO * jg : HALO * (jg + 1)],
                        kt[h][:, P * j : P * (j + 1)],
                        qt[h][:, M:MH],
                        start=(h == 0),
                        stop=(h == NH - 1),
                    )
                ej = store.tile([P, M], bf16, name=f"e{j}", tag="E", bufs=NJ)
                nc.scalar.activation(ej, ps, Exp, bias=bias_t)
                elist.append(ej)
                if jg == 7:
                    # tail exp for this group of 8 j-tiles
                    seg = slice(8 * HALO * g, 8 * HALO * (g + 1))
                    nc.scalar.activation(e8[:, seg], ps8g, Exp, bias=bias_t)

            # ---- phase C: Ru accumulation as one dependency-free matmul
            # stream (bf16, LDWEIGHTS pipelines away when back-to-back) ----
            for j in range(NJ):
                for c in range(NMAIN):
                    nc.tensor.matmul(
                        ru[c],
                        elist[j][:, P * c : P * (c + 1)],
                        vc[j],
                        start=(j == 0),
                        stop=(j == NJ - 1),
                    )
            for j in range(NJ):
                nc.tensor.matmul(
                    ru8,
                    e8[:, HALO * j : HALO * (j + 1)],
                    vc[j],
                    start=(j == 0),
                    stop=(j == NJ - 1),
                )

            # ---- normalize: R = Ru[:, :320] * mask / Ru[:, 320] ----
            rch = []
            for c in range(NMAIN):
                rec = small.tile([P, 1], f32, name=f"rec{c}", tag="rec")
                if c == 0:
                    # halo rows can have rowsum 0 (zero-padded queries on core
                    # 0); add (1-mask) so 1/den is finite, then zero via mask.
                    den = small.tile([P, 1], f32, name="den0", tag="den")
                    nc.vector.tensor_add(den, ru[c][:, NS * NV : NS * NV + 1], rminv)
                    nc.vector.reciprocal(rec, den)
                    nc.vector.tensor_mul(rec, rec, rmask)
                else:
                    nc.vector.reciprocal(rec, ru[c][:, NS * NV : NS * NV + 1])
                t = store.tile([P, NS * NV], bf16, name=f"r{c}", tag=f"r{c}")
                nc.vector.tensor_scalar_mul(t, ru[c][:, 0 : NS * NV], rec)
                rch.append(t)
            rec8 = small.tile([HALO, 1], f32, name="rec8", tag="rec8")
            nc.vector.reciprocal(rec8, ru8[:, NS * NV : NS * NV + 1])
            r8 = store.tile([HALO, NS * NV], bf16, name="r8", tag="r8")
            nc.vector.tensor_scalar_mul(r8, ru8[:, 0 : NS * NV], rec8)
            rch.append(r8)

            # ---- combine: out[p, v] = sum_s w_s R[128c + p + 8 - 2s, 80s + v] ----
            oall = small.tile([P, NMAIN, NV], f32, name="oall", tag="osb")
            for c in range(NMAIN):
                po = psA.tile([P, NV], f32, name=f"po{c}", tag="ps")
                for s in range(NS):
                    nc.tensor.matmul(
                        po,
                        sh1[:, P * s : P * (s + 1)],
                        rch[c][:, NV * s : NV * (s + 1)],
                        start=(s == 0),
                        stop=False,
                    )
                for s in range(NS):
                    nc.tensor.matmul(
                        po,
                        sh2[:, P * s : P * (s + 1)],
                        rch[c + 1][0:HALO, NV * s : NV * (s + 1)],
                        start=False,
                        stop=(s == NS - 1),
                    )
                nc.vector.tensor_copy(oall[:, c, :], po)
            nc.sync.dma_start(out=out_d.rearrange("c p v -> p c v"), in_=oall)

    nc.compile()
    return nc


def _get_nc():
    if "nc" not in _NC_CACHE:
        _install_axon_ntff_hook()
        _NC_CACHE["nc"] = _build_nc()
    return _NC_CACHE["nc"]


def _host_prep(x, x_key, x_value, W_qk, w_shift):
    bf = ml_dtypes.bfloat16
    x = np.ascontiguousarray(np.asarray(x, dtype=np.float32))
    x_key = np.ascontiguousarray(np.asarray(x_key, dtype=np.float32))
    x_value = np.ascontiguousarray(np.asarray(x_value, dtype=np.float32))
    W_qk = np.ascontiguousarray(np.asarray(W_qk, dtype=np.float32))
    w_shift = np.asarray(w_shift, dtype=np.float32)

    xkT = np.ascontiguousarray(x_key.T)                      # [Q, T]
    wT = np.ascontiguousarray(W_qk.T)                        # [Q, H]; wT[f,h]=W[h,f]

    vcomb = np.zeros((T, NS * NV + 1), np.float32)
    for s in range(NS):
        d = STEP * s
        vcomb[: T - d, NV * s : NV * (s + 1)] = x_value[d:, :]
    vcomb[:, NS * NV] = 1.0
    vcomb = np.ascontiguousarray(vcomb.astype(bf).reshape(NJ, P, NS * NV + 1))

    # shmat[s][k, p] = w_s * [k == p + 8 - 2s], k in [0, 136); packed in aux:
    # aux[:, 0:512] = sh1, aux[0:8, 512:1024] = sh2
    sh1 = np.zeros((P, NS, P), np.float32)
    sh2 = np.zeros((P, NS, P), np.float32)
    for s in range(NS):
        d = HALO - STEP * s
        for p in range(P):
            k = p + d
            if k < P:
                sh1[k, s, p] = w_shift[0, s]
            else:
                sh2[k - P, s, p] = w_shift[0, s]
    aux = np.concatenate(
        [sh1.reshape(P, NS * P), sh2.reshape(P, NS * P)], axis=1
    ).astype(bf)

    xpad = np.concatenate([np.zeros((HALO, Q), np.float32), x], axis=0)

    in_maps = []
    for d in range(NCORES):
        r0 = d * M
        xT = np.ascontiguousarray(xpad[r0 : r0 + MH].T)      # [Q, MH]
        masks = np.ones((P, 2), np.float32)
        masks[:, 1] = 0.0
        if d == 0:
            masks[:HALO, 0] = 0.0
            masks[:HALO, 1] = 1.0
        in_maps.append(
            {
                "xT": xT,
                "xkT": xkT,
                "wT": wT,
                "vcomb": vcomb,
                "aux": aux,
                "masks": masks,
            }
        )
    return in_maps


def kernel(x, x_key, x_value, W_qk, w_shift):
    global LAST_RESULT
    from concourse.bass_utils import run_bass_kernel_spmd

    nc = _get_nc()
    in_maps = _host_prep(x, x_key, x_value, W_qk, w_shift)
    res = run_bass_kernel_spmd(nc, in_maps, core_ids=list(range(NCORES)))
    LAST_RESULT = res
    out = np.concatenate(
        [res.results[d]["out"].reshape(M, NV) for d in range(NCORES)], axis=0
    )
    return out.astype(np.float32)


# revision 21
# speedup vs baseline: 1.3181x; 1.3181x over previous
"""Trainium2 Bass kernel for shifted sparse attention (nn_Attention_74672301408506).

Math (reference):
    q = x @ W.T ; k = x_key @ W.T ; att = softmax(q k^T)
    out[i, v] = sum_s w_s * sum_j att[i-2s, j] * x_value[j+2s, v]

Device algorithm (8 cores, query rows sharded, 512 rows per core, no halo):
    per core, query rows [r0, r0+512):
      qT[h, i], kT[h, j]     (float32r = tf32 matmuls, full-rate at N>=256)
      S^T[j, i] = kT^T q     (scores transposed: keys on partitions)
      E = exp(S - 100)       (bf16; softmax is shift-invariant)
      Ru[i, :] = E^T @ [V_0|V_1|V_2|V_3|ones]   (ones col = softmax denominator)
      R = Ru[:, :320] / Ru[:, 320]
      C[t, v] = sum_s w_s R[t - 2s, 80s + v]  for t in [0, 518)  (banded matmuls)
    C rows [0, 512) are complete except the first 6 rows, which miss the
    neighbor's contribution; rows [512, 518) are exactly that contribution for
    the next core. The host adds the 6-row overlaps when unsharding (exact).
"""

import os
import sys
import types

import numpy as np
import ml_dtypes

T = 4096
Q = 256
H = 256
NV = 80
NS = 4
STEP = 2
NCORES = 8
M = T // NCORES            # 512 rows per core
CSUB = 100.0               # global score shift before exp
P = 128
NJ = T // P                # 32 key tiles
NF = Q // P                # 2 feature tiles
NH = H // P                # 2 hidden tiles
NMAIN = M // P             # 4 i-chunks of 128
NVC = NS * NV + 1          # value width incl ones column
TAIL = (NS - 1) * STEP     # 6 overlap rows between neighboring cores


def _install_axon_ntff_hook():
    """bass_utils' trace path imports antenv.axon_hooks, which the agent image
    lacks; shim it and register the ctypes-based NTFF profiler hook."""
    if "antenv.axon_hooks" in sys.modules:
        return
    try:
        import antenv
    except ImportError:
        return
    mod = types.ModuleType("antenv.axon_hooks")
    mod._hook = None
    mod.set_axon_ntff_profile_hook = lambda h: setattr(mod, "_hook", h)
    mod.get_axon_ntff_profile_hook = lambda: mod._hook
    sys.modules["antenv.axon_hooks"] = mod
    antenv.axon_hooks = mod
    try:
        from trn_agent_boot import trn_boot

        so_path = "/opt/axon/libaxon_pjrt.so"
        if os.path.exists(so_path):
            mod.set_axon_ntff_profile_hook(trn_boot._ntff_profile_via_ctypes(so_path))
    except Exception:
        pass


_NC_CACHE = {}
LAST_RESULT = None


def _build_nc():
    import concourse.mybir as mybir
    import concourse.tile as tile
    from concourse import bacc

    f32 = mybir.dt.float32
    f32r = mybir.dt.float32r
    bf16 = mybir.dt.bfloat16
    Exp = mybir.ActivationFunctionType.Exp

    nc = bacc.Bacc(None, target_bir_lowering=False)

    xT_d = nc.dram_tensor("xT", [Q, M], f32r, kind="ExternalInput")
    xkT_d = nc.dram_tensor("xkT", [Q, T], f32r, kind="ExternalInput")
    wT_d = nc.dram_tensor("wT", [Q, H], f32r, kind="ExternalInput")
    # pre-tiled on host: row p holds tile-row p of every j-tile (big DMA descs)
    vc_d = nc.dram_tensor("vcomb", [P, NJ * NVC], bf16, kind="ExternalInput")
    aux_d = nc.dram_tensor("aux", [P, 8 * P + NS * TAIL], bf16, kind="ExternalInput")
    out_d = nc.dram_tensor("out", [NMAIN, P, NV], f32, kind="ExternalOutput")
    outt_d = nc.dram_tensor("outt", [TAIL, NV], f32, kind="ExternalOutput")

    with tile.TileContext(nc) as tc:
        with (
            tc.tile_pool(name="consts", bufs=1) as consts,
            tc.tile_pool(name="io", bufs=1) as io,
            tc.tile_pool(name="store", bufs=1) as store,
            tc.tile_pool(name="small", bufs=6) as small,
            tc.tile_pool(name="psA", bufs=3, space="PSUM") as psA,
            tc.tile_pool(name="psR", bufs=4, space="PSUM") as psR,
            tc.tile_pool(name="psW", bufs=1, space="PSUM") as psW,
        ):
            # ---- PE warmup: dummy matmuls while input DMAs stream, so HAM
            # reaches K=8/8 (2.4 GHz) before real work ----
            wu = consts.tile([P, 512], bf16, name="wu")
            nc.vector.memset(wu, 0.0)
            wups = psW.tile([P, 512], f32, name="wups", tag="wu")
            for i in range(14):
                nc.tensor.matmul(wups, wu[:, 0:P], wu, start=True, stop=True)

            # ---- inputs; spread across HWDGE rings (sync+scalar) and SWDGE ----
            aux = consts.tile([P, 8 * P + NS * TAIL], bf16, name="aux")
            nc.gpsimd.dma_start(out=aux, in_=aux_d[:, :])
            wt = []
            for f in range(NF):
                t = consts.tile([P, H], f32r, name=f"wt{f}", tag=f"wt{f}")
                eng = nc.sync if f == 0 else nc.scalar
                eng.dma_start(out=t, in_=wT_d[P * f : P * (f + 1), :])
                wt.append(t)
            xt = []
            for f in range(NF):
                t = consts.tile([P, M], f32r, name=f"xt{f}", tag=f"xt{f}")
                eng = nc.sync if f == 0 else nc.scalar
                eng.dma_start(out=t, in_=xT_d[P * f : P * (f + 1), :])
                xt.append(t)
            xkh = []
            for f in range(NF):
                halves = []
                for half in range(2):
                    t = io.tile(
                        [P, T // 2], f32r, name=f"xkh{f}_{half}", tag=f"xk{f}_{half}"
                    )
                    eng = nc.sync if f == 0 else nc.scalar
                    eng.dma_start(
                        out=t,
                        in_=xkT_d[
                            P * f : P * (f + 1), (T // 2) * half : (T // 2) * (half + 1)
                        ],
                    )
                    halves.append(t)
                xkh.append(halves)
            vcall = store.tile([P, NJ * NVC], bf16, name="vcall", tag="vc")
            nc.gpsimd.dma_start(out=vcall, in_=vc_d[:, :])
            vc = [vcall[:, NVC * j : NVC * (j + 1)] for j in range(NJ)]
            bias_t = consts.tile([P, 1], f32, name="bias_t")
            nc.vector.memset(bias_t, -CSUB)

            # ---- phase A: qT, kT ----
            qt = []
            for h in range(NH):
                ps = psA.tile([P, M], f32, name=f"qps{h}", tag="ps")
                for f in range(NF):
                    nc.tensor.matmul(
                        ps,
                        wt[f][:, P * h : P * (h + 1)],
                        xt[f],
                        start=(f == 0),
                        stop=(f == NF - 1),
                    )
                t = store.tile([P, M], f32r, name=f"qt{h}", tag=f"qt{h}")
                if h == 0:
                    nc.vector.tensor_copy(t, ps)
                else:
                    nc.scalar.copy(t, ps)
                qt.append(t)

            kt = []
            for h in range(NH):
                kt.append(store.tile([P, T], f32r, name=f"kt{h}", tag=f"kt{h}"))
            NKC = 8  # 512-wide key chunks
            for jc in range(NKC):
                xkc = [
                    xkh[f][jc // 4][:, 512 * (jc % 4) : 512 * (jc % 4 + 1)]
                    for f in range(NF)
                ]
                for h in range(NH):
                    ps = psA.tile([P, 512], f32, name=f"kps{jc}_{h}", tag="ps")
                    for f in range(NF):
                        nc.tensor.matmul(
                            ps,
                            wt[f][:, P * h : P * (h + 1)],
                            xkc[f],
                            start=(f == 0),
                            stop=(f == NF - 1),
                        )
                    dst = kt[h][:, 512 * jc : 512 * (jc + 1)]
                    if (jc + h) % 2 == 0:
                        nc.vector.tensor_copy(dst, ps)
                    else:
                        nc.scalar.copy(dst, ps)

            # ---- phases B+C: scores^T -> exp -> Ru, interleaved with a lag of
            # DELAY j-tiles so Ru matmuls never wait on the exp just issued ----
            ru = []
            for c in range(NMAIN):
                ru.append(psR.tile([P, NVC], f32, name=f"ru{c}", tag="ru"))
            elist = []
            DELAY = 2

            def ru_step(j):
                for c in range(NMAIN):
                    nc.tensor.matmul(
                        ru[c],
                        elist[j][:, P * c : P * (c + 1)],
                        vc[j],
                        start=(j == 0),
                        stop=(j == NJ - 1),
                    )

            for j in range(NJ):
                ps = psA.tile([P, M], f32, name=f"sps{j}", tag="ps")
                for h in range(NH):
                    nc.tensor.matmul(
                        ps,
                        kt[h][:, P * j : P * (j + 1)],
                        qt[h],
                        start=(h == 0),
                        stop=(h == NH - 1),
                    )
                ej = store.tile([P, M], bf16, name=f"e{j}", tag="E", bufs=NJ)
                nc.scalar.activation(ej, ps, Exp, bias=bias_t)
                elist.append(ej)
                if j >= DELAY:
                    ru_step(j - DELAY)
            for j in range(NJ - DELAY, NJ):
                ru_step(j)

            # ---- normalize + combine, chunk by chunk ----
            # aux layout (bf16), all [128, 128] banded matrices with w_s baked:
            #   A1 = aux[:, 128s:128(s+1)]        k == p - 2s       (own chunk)
            #   A2 = aux[:, 512+128s:512+128(s+1)] k == 128 + p - 2s (prev chunk)
            #   A3 = aux[:, 1024+6s:1024+6(s+1)]  k == 128 + t' - 2s (tail rows)
            rch = []
            for c in range(NMAIN):
                rec = small.tile([P, 1], f32, name=f"rec{c}", tag="rec")
                nc.vector.reciprocal(rec, ru[c][:, NS * NV : NVC])
                t = store.tile([P, NS * NV], bf16, name=f"r{c}", tag=f"r{c}")
                nc.vector.tensor_scalar_mul(t, ru[c][:, 0 : NS * NV], rec)
                rch.append(t)

            oall = small.tile([P, NMAIN, NV], f32, name="oall", tag="osb")
            for c in range(NMAIN):
                po = psA.tile([P, NV], f32, name=f"po{c}", tag="ps")
                nmm = NS + (NS - 1 if c > 0 else 0)
                i = 0
                for s in range(NS):
                    nc.tensor.matmul(
                        po,
                        aux[:, P * s : P * (s + 1)],
                        rch[c][:, NV * s : NV * (s + 1)],
                        start=(i == 0),
                        stop=(i == nmm - 1),
                    )
                    i += 1
                if c > 0:
                    for s in range(1, NS):
                        nc.tensor.matmul(
                            po,
                            aux[:, 4 * P + P * s : 4 * P + P * (s + 1)],
                            rch[c - 1][:, NV * s : NV * (s + 1)],
                            start=False,
                            stop=(i == nmm - 1),
                        )
                        i += 1
                nc.vector.tensor_copy(oall[:, c, :], po)
            nc.sync.dma_start(out=out_d.rearrange("c p v -> p c v"), in_=oall)

            # tail rows [512, 518): next core's missing contribution
            pot = psA.tile([TAIL, NV], f32, name="pot", tag="ps")
            for s in range(1, NS):
                nc.tensor.matmul(
                    pot,
                    aux[:, 8 * P + TAIL * s : 8 * P + TAIL * (s + 1)],
                    rch[NMAIN - 1][:, NV * s : NV * (s + 1)],
                    start=(s == 1),
                    stop=(s == NS - 1),
                )
            ot = small.tile([TAIL, NV], f32, name="ot", tag="ot")
            nc.vector.tensor_copy(ot, pot)
            nc.sync.dma_start(out=outt_d[:, :], in_=ot)

    nc.compile()
    return nc


def _get_nc():
    if "nc" not in _NC_CACHE:
        _install_axon_ntff_hook()
        _NC_CACHE["nc"] = _build_nc()
    return _NC_CACHE["nc"]


def _host_prep(x, x_key, x_value, W_qk, w_shift):
    bf = ml_dtypes.bfloat16
    x = np.ascontiguousarray(np.asarray(x, dtype=np.float32))
    x_key = np.ascontiguousarray(np.asarray(x_key, dtype=np.float32))
    x_value = np.ascontiguousarray(np.asarray(x_value, dtype=np.float32))
    W_qk = np.ascontiguousarray(np.asarray(W_qk, dtype=np.float32))
    w_shift = np.asarray(w_shift, dtype=np.float32)

    xkT = np.ascontiguousarray(x_key.T)                      # [Q, T]
    wT = np.ascontiguousarray(W_qk.T)                        # [Q, H]; wT[f,h]=W[h,f]

    vcomb = np.zeros((T, NVC), np.float32)
    for s in range(NS):
        d = STEP * s
        vcomb[: T - d, NV * s : NV * (s + 1)] = x_value[d:, :]
    vcomb[:, NS * NV] = 1.0
    # pre-tile: [T, NVC] -> [NJ, P, NVC] -> [P, NJ*NVC] so each SBUF partition
    # line is one contiguous DMA descriptor
    vcomb = np.ascontiguousarray(
        vcomb.astype(bf).reshape(NJ, P, NVC).transpose(1, 0, 2).reshape(P, NJ * NVC)
    )

    # combine matrices (see aux layout comment in _build_nc)
    aux = np.zeros((P, 8 * P + NS * TAIL), np.float32)
    for s in range(NS):
        w = w_shift[0, s]
        for p in range(P):
            k = p - STEP * s
            if 0 <= k < P:
                aux[k, P * s + p] = w                      # A1
            kk = P + p - STEP * s
            if 0 <= kk < P:
                aux[kk, 4 * P + P * s + p] = w             # A2 (prev chunk)
        if s >= 1:
            for tp in range(TAIL):
                k = P + tp - STEP * s
                if 0 <= k < P:
                    aux[k, 8 * P + TAIL * s + tp] = w      # A3 (tail rows)
    aux = aux.astype(bf)

    in_maps = []
    for d in range(NCORES):
        r0 = d * M
        xT = np.ascontiguousarray(x[r0 : r0 + M].T)          # [Q, M]
        in_maps.append(
            {"xT": xT, "xkT": xkT, "wT": wT, "vcomb": vcomb, "aux": aux}
        )
    return in_maps


def kernel(x, x_key, x_value, W_qk, w_shift):
    global LAST_RESULT
    from concourse.bass_utils import run_bass_kernel_spmd

    nc = _get_nc()
    in_maps = _host_prep(x, x_key, x_value, W_qk, w_shift)
    res = run_bass_kernel_spmd(nc, in_maps, core_ids=list(range(NCORES)))
    LAST_RESULT = res
    out = np.concatenate(
        [res.results[d]["out"].reshape(M, NV) for d in range(NCORES)], axis=0
    )
    # add the 6-row cross-core overlap contributions
    for d in range(NCORES - 1):
        out[M * (d + 1) : M * (d + 1) + TAIL] += res.results[d]["outt"]
    return out.astype(np.float32)


# revision 22
# speedup vs baseline: 1.4043x; 1.0654x over previous
"""Trainium2 Bass kernel for shifted sparse attention (nn_Attention_74672301408506).

Math (reference):
    q = x @ W.T ; k = x_key @ W.T ; att = softmax(q k^T)
    out[i, v] = sum_s w_s * sum_j att[i-2s, j] * x_value[j+2s, v]

Device algorithm (8 cores, query rows sharded, 512 rows per core, no halo):
    per core, query rows [r0, r0+512):
      qT[h, i], kT[h, j]     (float32r = tf32 matmuls, full-rate at N>=256)
      S^T[j, i] = kT^T q     (scores transposed: keys on partitions)
      E = exp(S - 100)       (bf16; softmax is shift-invariant)
      Ru[i, :] = E^T @ [V_0|V_1|V_2|V_3|ones]   (ones col = softmax denominator)
      R = Ru[:, :320] / Ru[:, 320]
      C[t, v] = sum_s w_s R[t - 2s, 80s + v]  for t in [0, 518)  (banded matmuls)
    C rows [0, 512) are complete except the first 6 rows, which miss the
    neighbor's contribution; rows [512, 518) are exactly that contribution for
    the next core. The host adds the 6-row overlaps when unsharding (exact).
"""

import os
import sys
import types

import numpy as np
import ml_dtypes

T = 4096
Q = 256
H = 256
NV = 80
NS = 4
STEP = 2
NCORES = 8
M = T // NCORES            # 512 rows per core
CSUB = 100.0               # global score shift before exp
P = 128
NJ = T // P                # 32 key tiles
NF = Q // P                # 2 feature tiles
NH = H // P                # 2 hidden tiles
NMAIN = M // P             # 4 i-chunks of 128
NVC = NS * NV + 1          # value width incl ones column
TAIL = (NS - 1) * STEP     # 6 overlap rows between neighboring cores


def _install_axon_ntff_hook():
    """bass_utils' trace path imports antenv.axon_hooks, which the agent image
    lacks; shim it and register the ctypes-based NTFF profiler hook."""
    if "antenv.axon_hooks" in sys.modules:
        return
    try:
        import antenv
    except ImportError:
        return
    mod = types.ModuleType("antenv.axon_hooks")
    mod._hook = None
    mod.set_axon_ntff_profile_hook = lambda h: setattr(mod, "_hook", h)
    mod.get_axon_ntff_profile_hook = lambda: mod._hook
    sys.modules["antenv.axon_hooks"] = mod
    antenv.axon_hooks = mod
    try:
        from trn_agent_boot import trn_boot

        so_path = "/opt/axon/libaxon_pjrt.so"
        if os.path.exists(so_path):
            mod.set_axon_ntff_profile_hook(trn_boot._ntff_profile_via_ctypes(so_path))
    except Exception:
        pass


_NC_CACHE = {}
LAST_RESULT = None


def _build_nc():
    import concourse.mybir as mybir
    import concourse.tile as tile
    from concourse import bacc

    f32 = mybir.dt.float32
    f32r = mybir.dt.float32r
    bf16 = mybir.dt.bfloat16
    Exp = mybir.ActivationFunctionType.Exp

    nc = bacc.Bacc(None, target_bir_lowering=False)

    xT_d = nc.dram_tensor("xT", [Q, M], f32r, kind="ExternalInput")
    xkT_d = nc.dram_tensor("xkT", [Q, T], f32r, kind="ExternalInput")
    wT_d = nc.dram_tensor("wT", [Q, H], f32r, kind="ExternalInput")
    # pre-tiled on host: row p holds tile-row p of every j-tile (big DMA descs)
    vc_d = nc.dram_tensor("vcomb", [P, NJ * NVC], bf16, kind="ExternalInput")
    aux_d = nc.dram_tensor("aux", [P, 8 * P + NS * TAIL], bf16, kind="ExternalInput")
    out_d = nc.dram_tensor("out", [NMAIN, P, NV], f32, kind="ExternalOutput")
    outt_d = nc.dram_tensor("outt", [TAIL, NV], f32, kind="ExternalOutput")

    with tile.TileContext(nc) as tc:
        with (
            tc.tile_pool(name="consts", bufs=1) as consts,
            tc.tile_pool(name="io", bufs=1) as io,
            tc.tile_pool(name="store", bufs=1) as store,
            tc.tile_pool(name="small", bufs=6) as small,
            tc.tile_pool(name="psA", bufs=3, space="PSUM") as psA,
            tc.tile_pool(name="psR", bufs=4, space="PSUM") as psR,
            tc.tile_pool(name="psW", bufs=1, space="PSUM") as psW,
        ):
            # ---- PE warmup: dummy matmuls while input DMAs stream, so HAM
            # reaches K=8/8 (2.4 GHz) before real work ----
            wu = consts.tile([P, 512], bf16, name="wu")
            nc.vector.memset(wu, 0.0)
            wups = psW.tile([P, 512], f32, name="wups", tag="wu")
            for i in range(14):
                nc.tensor.matmul(wups, wu[:, 0:P], wu, start=True, stop=True)

            # ---- inputs; spread across HWDGE rings (sync+scalar) and SWDGE ----
            aux = consts.tile([P, 8 * P + NS * TAIL], bf16, name="aux")
            nc.gpsimd.dma_start(out=aux, in_=aux_d[:, :])
            wt = []
            for f in range(NF):
                t = consts.tile([P, H], f32r, name=f"wt{f}", tag=f"wt{f}")
                eng = nc.sync if f == 0 else nc.scalar
                eng.dma_start(out=t, in_=wT_d[P * f : P * (f + 1), :])
                wt.append(t)
            xt = []
            for f in range(NF):
                t = consts.tile([P, M], f32r, name=f"xt{f}", tag=f"xt{f}")
                eng = nc.sync if f == 0 else nc.scalar
                eng.dma_start(out=t, in_=xT_d[P * f : P * (f + 1), :])
                xt.append(t)
            xkh = []
            for f in range(NF):
                quarters = []
                for qq in range(4):
                    t = io.tile(
                        [P, T // 4], f32r, name=f"xkh{f}_{qq}", tag=f"xk{f}_{qq}"
                    )
                    eng = nc.sync if f == 0 else nc.scalar
                    eng.dma_start(
                        out=t,
                        in_=xkT_d[
                            P * f : P * (f + 1), (T // 4) * qq : (T // 4) * (qq + 1)
                        ],
                    )
                    quarters.append(t)
                xkh.append(quarters)
            vcall = store.tile([P, NJ * NVC], bf16, name="vcall", tag="vc")
            nc.gpsimd.dma_start(out=vcall, in_=vc_d[:, :])
            vc = [vcall[:, NVC * j : NVC * (j + 1)] for j in range(NJ)]
            bias_t = consts.tile([P, 1], f32, name="bias_t")
            nc.vector.memset(bias_t, -CSUB)

            # ---- phase A: qT, kT ----
            qt = []
            for h in range(NH):
                ps = psA.tile([P, M], f32, name=f"qps{h}", tag="ps")
                for f in range(NF):
                    nc.tensor.matmul(
                        ps,
                        wt[f][:, P * h : P * (h + 1)],
                        xt[f],
                        start=(f == 0),
                        stop=(f == NF - 1),
                    )
                t = store.tile([P, M], f32r, name=f"qt{h}", tag=f"qt{h}")
                if h == 0:
                    nc.vector.tensor_copy(t, ps)
                else:
                    nc.scalar.copy(t, ps)
                qt.append(t)

            # ---- merged pipeline: per 512-wide key chunk jc, compute kT(jc),
            # then S^T/exp for its 4 j-tiles, with Ru lagging DELAY j-tiles so
            # its matmuls never wait on a just-issued exp ----
            kt = []
            for h in range(NH):
                kt.append(store.tile([P, T], f32r, name=f"kt{h}", tag=f"kt{h}"))
            ru = []
            for c in range(NMAIN):
                ru.append(psR.tile([P, NVC], f32, name=f"ru{c}", tag="ru"))
            elist = []
            DELAY = 2

            def ru_step(j):
                for c in range(NMAIN):
                    nc.tensor.matmul(
                        ru[c],
                        elist[j][:, P * c : P * (c + 1)],
                        vc[j],
                        start=(j == 0),
                        stop=(j == NJ - 1),
                    )

            NKC = 8  # 512-wide key chunks
            for jc in range(NKC):
                xkc = [
                    xkh[f][jc // 2][:, 512 * (jc % 2) : 512 * (jc % 2 + 1)]
                    for f in range(NF)
                ]
                for h in range(NH):
                    ps = psA.tile([P, 512], f32, name=f"kps{jc}_{h}", tag="ps")
                    for f in range(NF):
                        nc.tensor.matmul(
                            ps,
                            wt[f][:, P * h : P * (h + 1)],
                            xkc[f],
                            start=(f == 0),
                            stop=(f == NF - 1),
                        )
                    dst = kt[h][:, 512 * jc : 512 * (jc + 1)]
                    if (jc + h) % 2 == 0:
                        nc.vector.tensor_copy(dst, ps)
                    else:
                        nc.scalar.copy(dst, ps)
                for j in range(4 * jc, 4 * (jc + 1)):
                    ps = psA.tile([P, M], f32, name=f"sps{j}", tag="ps")
                    for h in range(NH):
                        nc.tensor.matmul(
                            ps,
                            kt[h][:, P * j : P * (j + 1)],
                            qt[h],
                            start=(h == 0),
                            stop=(h == NH - 1),
                        )
                    ej = store.tile([P, M], bf16, name=f"e{j}", tag="E", bufs=NJ)
                    nc.scalar.activation(ej, ps, Exp, bias=bias_t)
                    elist.append(ej)
                    if j >= DELAY:
                        ru_step(j - DELAY)
            for j in range(NJ - DELAY, NJ):
                ru_step(j)

            # ---- normalize + combine, chunk by chunk ----
            # aux layout (bf16), all [128, 128] banded matrices with w_s baked:
            #   A1 = aux[:, 128s:128(s+1)]        k == p - 2s       (own chunk)
            #   A2 = aux[:, 512+128s:512+128(s+1)] k == 128 + p - 2s (prev chunk)
            #   A3 = aux[:, 1024+6s:1024+6(s+1)]  k == 128 + t' - 2s (tail rows)
            rch = []
            for c in range(NMAIN):
                rec = small.tile([P, 1], f32, name=f"rec{c}", tag="rec")
                nc.vector.reciprocal(rec, ru[c][:, NS * NV : NVC])
                t = store.tile([P, NS * NV], bf16, name=f"r{c}", tag=f"r{c}")
                nc.vector.tensor_scalar_mul(t, ru[c][:, 0 : NS * NV], rec)
                rch.append(t)

            oall = small.tile([P, NMAIN, NV], f32, name="oall", tag="osb")
            for c in range(NMAIN):
                po = psA.tile([P, NV], f32, name=f"po{c}", tag="ps")
                nmm = NS + (NS - 1 if c > 0 else 0)
                i = 0
                for s in range(NS):
                    nc.tensor.matmul(
                        po,
                        aux[:, P * s : P * (s + 1)],
                        rch[c][:, NV * s : NV * (s + 1)],
                        start=(i == 0),
                        stop=(i == nmm - 1),
                    )
                    i += 1
                if c > 0:
                    for s in range(1, NS):
                        nc.tensor.matmul(
                            po,
                            aux[:, 4 * P + P * s : 4 * P + P * (s + 1)],
                            rch[c - 1][:, NV * s : NV * (s + 1)],
                            start=False,
                            stop=(i == nmm - 1),
                        )
                        i += 1
                nc.vector.tensor_copy(oall[:, c, :], po)
            nc.sync.dma_start(out=out_d.rearrange("c p v -> p c v"), in_=oall)

            # tail rows [512, 518): next core's missing contribution
            pot = psA.tile([TAIL, NV], f32, name="pot", tag="ps")
            for s in range(1, NS):
                nc.tensor.matmul(
                    pot,
                    aux[:, 8 * P + TAIL * s : 8 * P + TAIL * (s + 1)],
                    rch[NMAIN - 1][:, NV * s : NV * (s + 1)],
                    start=(s == 1),
                    stop=(s == NS - 1),
                )
            ot = small.tile([TAIL, NV], f32, name="ot", tag="ot")
            nc.vector.tensor_copy(ot, pot)
            nc.sync.dma_start(out=outt_d[:, :], in_=ot)

    nc.compile()
    return nc


def _get_nc():
    if "nc" not in _NC_CACHE:
        _install_axon_ntff_hook()
        _NC_CACHE["nc"] = _build_nc()
    return _NC_CACHE["nc"]


def _host_prep(x, x_key, x_value, W_qk, w_shift):
    bf = ml_dtypes.bfloat16
    x = np.ascontiguousarray(np.asarray(x, dtype=np.float32))
    x_key = np.ascontiguousarray(np.asarray(x_key, dtype=np.float32))
    x_value = np.ascontiguousarray(np.asarray(x_value, dtype=np.float32))
    W_qk = np.ascontiguousarray(np.asarray(W_qk, dtype=np.float32))
    w_shift = np.asarray(w_shift, dtype=np.float32)

    xkT = np.ascontiguousarray(x_key.T)                      # [Q, T]
    wT = np.ascontiguousarray(W_qk.T)                        # [Q, H]; wT[f,h]=W[h,f]

    vcomb = np.zeros((T, NVC), np.float32)
    for s in range(NS):
        d = STEP * s
        vcomb[: T - d, NV * s : NV * (s + 1)] = x_value[d:, :]
    vcomb[:, NS * NV] = 1.0
    # pre-tile: [T, NVC] -> [NJ, P, NVC] -> [P, NJ*NVC] so each SBUF partition
    # line is one contiguous DMA descriptor
    vcomb = np.ascontiguousarray(
        vcomb.astype(bf).reshape(NJ, P, NVC).transpose(1, 0, 2).reshape(P, NJ * NVC)
    )

    # combine matrices (see aux layout comment in _build_nc)
    aux = np.zeros((P, 8 * P + NS * TAIL), np.float32)
    for s in range(NS):
        w = w_shift[0, s]
        for p in range(P):
            k = p - STEP * s
            if 0 <= k < P:
                aux[k, P * s + p] = w                      # A1
            kk = P + p - STEP * s
            if 0 <= kk < P:
                aux[kk, 4 * P + P * s + p] = w             # A2 (prev chunk)
        if s >= 1:
            for tp in range(TAIL):
                k = P + tp - STEP * s
                if 0 <= k < P:
                    aux[k, 8 * P + TAIL * s + tp] = w      # A3 (tail rows)
    aux = aux.astype(bf)

    in_maps = []
    for d in range(NCORES):
        r0 = d * M
        xT = np.ascontiguousarray(x[r0 : r0 + M].T)          # [Q, M]
        in_maps.append(
            {"xT": xT, "xkT": xkT, "wT": wT, "vcomb": vcomb, "aux": aux}
        )
    return in_maps


def kernel(x, x_key, x_value, W_qk, w_shift):
    global LAST_RESULT
    from concourse.bass_utils import run_bass_kernel_spmd

    nc = _get_nc()
    in_maps = _host_prep(x, x_key, x_value, W_qk, w_shift)
    res = run_bass_kernel_spmd(nc, in_maps, core_ids=list(range(NCORES)))
    LAST_RESULT = res
    out = np.concatenate(
        [res.results[d]["out"].reshape(M, NV) for d in range(NCORES)], axis=0
    )
    # add the 6-row cross-core overlap contributions
    for d in range(NCORES - 1):
        out[M * (d + 1) : M * (d + 1) + TAIL] += res.results[d]["outt"]
    return out.astype(np.float32)


# revision 24
# speedup vs baseline: 1.5572x; 1.1089x over previous
"""Trainium2 Bass kernel for shifted sparse attention (nn_Attention_74672301408506).

Math (reference):
    q = x @ W.T ; k = x_key @ W.T ; att = softmax(q k^T)
    out[i, v] = sum_s w_s * sum_j att[i-2s, j] * x_value[j+2s, v]

Device algorithm (8 cores, query rows sharded, 512 rows per core, no halo):
    per core, query rows [r0, r0+512):
      qT[h, i], kT[h, j]     (float32r = tf32 matmuls, full-rate at N>=256)
      S^T[j, i] = kT^T q     (scores transposed: keys on partitions)
      E = exp(S - 100)       (bf16; softmax is shift-invariant)
      Ru[i, :] = E^T @ [V_0|V_1|V_2|V_3|ones]   (ones col = softmax denominator)
      R = Ru[:, :320] / Ru[:, 320]
      C[t, v] = sum_s w_s R[t - 2s, 80s + v]  for t in [0, 518)  (banded matmuls)
    C rows [0, 512) are complete except the first 6 rows, which miss the
    neighbor's contribution; rows [512, 518) are exactly that contribution for
    the next core. The host adds the 6-row overlaps when unsharding (exact).
"""

import os
import sys
import types

import numpy as np
import ml_dtypes

T = 4096
Q = 256
H = 256
NV = 80
NS = 4
STEP = 2
NCORES = 8
M = T // NCORES            # 512 rows per core
CSUB = 100.0               # global score shift before exp
P = 128
NJ = T // P                # 32 key tiles
NF = Q // P                # 2 feature tiles
NH = H // P                # 2 hidden tiles
NMAIN = M // P             # 4 i-chunks of 128
NVC = NS * NV + 1          # value width incl ones column
TAIL = (NS - 1) * STEP     # 6 overlap rows between neighboring cores


def _install_axon_ntff_hook():
    """bass_utils' trace path imports antenv.axon_hooks, which the agent image
    lacks; shim it and register the ctypes-based NTFF profiler hook."""
    if "antenv.axon_hooks" in sys.modules:
        return
    try:
        import antenv
    except ImportError:
        return
    mod = types.ModuleType("antenv.axon_hooks")
    mod._hook = None
    mod.set_axon_ntff_profile_hook = lambda h: setattr(mod, "_hook", h)
    mod.get_axon_ntff_profile_hook = lambda: mod._hook
    sys.modules["antenv.axon_hooks"] = mod
    antenv.axon_hooks = mod
    try:
        from trn_agent_boot import trn_boot

        so_path = "/opt/axon/libaxon_pjrt.so"
        if os.path.exists(so_path):
            mod.set_axon_ntff_profile_hook(trn_boot._ntff_profile_via_ctypes(so_path))
    except Exception:
        pass


_NC_CACHE = {}
LAST_RESULT = None


def _build_nc():
    import concourse.mybir as mybir
    import concourse.tile as tile
    from concourse import bacc

    f32 = mybir.dt.float32
    f32r = mybir.dt.float32r
    bf16 = mybir.dt.bfloat16
    Exp = mybir.ActivationFunctionType.Exp

    nc = bacc.Bacc(None, target_bir_lowering=False)

    xT_d = nc.dram_tensor("xT", [Q, M], f32r, kind="ExternalInput")
    xkT_d = nc.dram_tensor("xkT", [Q, T], f32r, kind="ExternalInput")
    wT_d = nc.dram_tensor("wT", [Q, H], f32r, kind="ExternalInput")
    # pre-tiled on host: row p holds tile-row p of every j-tile (big DMA descs)
    vc_d = nc.dram_tensor("vcomb", [P, NJ * NVC], bf16, kind="ExternalInput")
    aux_d = nc.dram_tensor("aux", [P, 8 * P + NS * TAIL], bf16, kind="ExternalInput")
    out_d = nc.dram_tensor("out", [NMAIN, P, NV], f32, kind="ExternalOutput")
    outt_d = nc.dram_tensor("outt", [TAIL, NV], f32, kind="ExternalOutput")

    with tile.TileContext(nc) as tc:
        with (
            tc.tile_pool(name="consts", bufs=1) as consts,
            tc.tile_pool(name="io", bufs=1) as io,
            tc.tile_pool(name="store", bufs=1) as store,
            tc.tile_pool(name="small", bufs=6) as small,
            tc.tile_pool(name="psA", bufs=3, space="PSUM") as psA,
            tc.tile_pool(name="psR", bufs=4, space="PSUM") as psR,
            tc.tile_pool(name="psW", bufs=1, space="PSUM") as psW,
        ):
            # ---- PE warmup: dummy matmuls while input DMAs stream, so HAM
            # reaches K=8/8 (2.4 GHz) before real work ----
            wu = consts.tile([P, 512], bf16, name="wu")
            nc.vector.memset(wu, 0.0)
            wups = psW.tile([P, 512], f32, name="wups", tag="wu")
            for i in range(14):
                nc.tensor.matmul(wups, wu[:, 0:P], wu, start=True, stop=True)

            # ---- inputs; spread across HWDGE rings (sync+scalar) and SWDGE ----
            aux = consts.tile([P, 8 * P + NS * TAIL], bf16, name="aux")
            nc.gpsimd.dma_start(out=aux, in_=aux_d[:, :])
            wt = []
            for f in range(NF):
                t = consts.tile([P, H], f32r, name=f"wt{f}", tag=f"wt{f}")
                eng = nc.sync if f == 0 else nc.scalar
                eng.dma_start(out=t, in_=wT_d[P * f : P * (f + 1), :])
                wt.append(t)
            xt = []
            for f in range(NF):
                t = consts.tile([P, M], f32r, name=f"xt{f}", tag=f"xt{f}")
                eng = nc.sync if f == 0 else nc.scalar
                eng.dma_start(out=t, in_=xT_d[P * f : P * (f + 1), :])
                xt.append(t)
            xkh = []
            for f in range(NF):
                quarters = []
                for qq in range(4):
                    t = io.tile(
                        [P, T // 4], f32r, name=f"xkh{f}_{qq}", tag=f"xk{f}_{qq}"
                    )
                    eng = nc.sync if f == 0 else nc.scalar
                    eng.dma_start(
                        out=t,
                        in_=xkT_d[
                            P * f : P * (f + 1), (T // 4) * qq : (T // 4) * (qq + 1)
                        ],
                    )
                    quarters.append(t)
                xkh.append(quarters)
            # vcomb in 4 group-DMAs: group 0 early on the gpsimd ring; groups
            # 1-3 ride the sync/scalar HWDGE rings BEHIND the xk quarters (ring
            # FIFO order keeps them off the critical early HBM window)
            vcall = store.tile([P, NJ * NVC], bf16, name="vcall", tag="vc")
            GW = 8 * NVC
            for g, eng in [(0, nc.gpsimd), (1, nc.sync), (2, nc.scalar), (3, nc.sync)]:
                eng.dma_start(
                    out=vcall[:, GW * g : GW * (g + 1)],
                    in_=vc_d[:, GW * g : GW * (g + 1)],
                )
            vc = [vcall[:, NVC * j : NVC * (j + 1)] for j in range(NJ)]
            bias_t = consts.tile([P, 1], f32, name="bias_t")
            nc.vector.memset(bias_t, -CSUB)

            # ---- phase A: qT, kT ----
            qt = []
            for h in range(NH):
                ps = psA.tile([P, M], f32, name=f"qps{h}", tag="ps")
                for f in range(NF):
                    nc.tensor.matmul(
                        ps,
                        wt[f][:, P * h : P * (h + 1)],
                        xt[f],
                        start=(f == 0),
                        stop=(f == NF - 1),
                    )
                t = store.tile([P, M], f32r, name=f"qt{h}", tag=f"qt{h}")
                if h == 0:
                    nc.vector.tensor_copy(t, ps)
                else:
                    nc.scalar.copy(t, ps)
                qt.append(t)

            # ---- merged pipeline: per 512-wide key chunk jc, compute kT(jc),
            # then S^T/exp for its 4 j-tiles, with Ru lagging DELAY j-tiles so
            # its matmuls never wait on a just-issued exp ----
            kt = []
            for h in range(NH):
                kt.append(store.tile([P, T], f32r, name=f"kt{h}", tag=f"kt{h}"))
            ru = []
            for c in range(NMAIN):
                ru.append(psR.tile([P, NVC], f32, name=f"ru{c}", tag="ru"))
            elist = []
            DELAY = 2

            def ru_step(j):
                for c in range(NMAIN):
                    nc.tensor.matmul(
                        ru[c],
                        elist[j][:, P * c : P * (c + 1)],
                        vc[j],
                        start=(j == 0),
                        stop=(j == NJ - 1),
                    )

            NKC = 8  # 512-wide key chunks
            for jc in range(NKC):
                xkc = [
                    xkh[f][jc // 2][:, 512 * (jc % 2) : 512 * (jc % 2 + 1)]
                    for f in range(NF)
                ]
                for h in range(NH):
                    ps = psA.tile([P, 512], f32, name=f"kps{jc}_{h}", tag="ps")
                    for f in range(NF):
                        nc.tensor.matmul(
                            ps,
                            wt[f][:, P * h : P * (h + 1)],
                            xkc[f],
                            start=(f == 0),
                            stop=(f == NF - 1),
                        )
                    dst = kt[h][:, 512 * jc : 512 * (jc + 1)]
                    if (jc + h) % 2 == 0:
                        nc.vector.tensor_copy(dst, ps)
                    else:
                        nc.scalar.copy(dst, ps)
                for j in range(4 * jc, 4 * (jc + 1)):
                    ps = psA.tile([P, M], f32, name=f"sps{j}", tag="ps")
                    for h in range(NH):
                        nc.tensor.matmul(
                            ps,
                            kt[h][:, P * j : P * (j + 1)],
                            qt[h],
                            start=(h == 0),
                            stop=(h == NH - 1),
                        )
                    ej = store.tile([P, M], bf16, name=f"e{j}", tag="E", bufs=NJ)
                    nc.scalar.activation(ej, ps, Exp, bias=bias_t)
                    elist.append(ej)
                    if j >= DELAY:
                        ru_step(j - DELAY)
            for j in range(NJ - DELAY, NJ):
                ru_step(j)

            # ---- normalize + combine, chunk by chunk ----
            # aux layout (bf16), all [128, 128] banded matrices with w_s baked:
            #   A1 = aux[:, 128s:128(s+1)]        k == p - 2s       (own chunk)
            #   A2 = aux[:, 512+128s:512+128(s+1)] k == 128 + p - 2s (prev chunk)
            #   A3 = aux[:, 1024+6s:1024+6(s+1)]  k == 128 + t' - 2s (tail rows)
            rch = []
            for c in range(NMAIN):
                rec = small.tile([P, 1], f32, name=f"rec{c}", tag="rec")
                nc.vector.reciprocal(rec, ru[c][:, NS * NV : NVC])
                t = store.tile([P, NS * NV], bf16, name=f"r{c}", tag=f"r{c}")
                nc.vector.tensor_scalar_mul(t, ru[c][:, 0 : NS * NV], rec)
                rch.append(t)

            oall = small.tile([P, NMAIN, NV], f32, name="oall", tag="osb")
            for c in range(NMAIN):
                po = psA.tile([P, NV], f32, name=f"po{c}", tag="ps")
                nmm = NS + (NS - 1 if c > 0 else 0)
                i = 0
                for s in range(NS):
                    nc.tensor.matmul(
                        po,
                        aux[:, P * s : P * (s + 1)],
                        rch[c][:, NV * s : NV * (s + 1)],
                        start=(i == 0),
                        stop=(i == nmm - 1),
                    )
                    i += 1
                if c > 0:
                    for s in range(1, NS):
                        nc.tensor.matmul(
                            po,
                            aux[:, 4 * P + P * s : 4 * P + P * (s + 1)],
                            rch[c - 1][:, NV * s : NV * (s + 1)],
                            start=False,
                            stop=(i == nmm - 1),
                        )
                        i += 1
                nc.vector.tensor_copy(oall[:, c, :], po)
            nc.sync.dma_start(out=out_d.rearrange("c p v -> p c v"), in_=oall)

            # tail rows [512, 518): next core's missing contribution
            pot = psA.tile([TAIL, NV], f32, name="pot", tag="ps")
            for s in range(1, NS):
                nc.tensor.matmul(
                    pot,
                    aux[:, 8 * P + TAIL * s : 8 * P + TAIL * (s + 1)],
                    rch[NMAIN - 1][:, NV * s : NV * (s + 1)],
                    start=(s == 1),
                    stop=(s == NS - 1),
                )
            ot = small.tile([TAIL, NV], f32, name="ot", tag="ot")
            nc.vector.tensor_copy(ot, pot)
            nc.sync.dma_start(out=outt_d[:, :], in_=ot)

    nc.compile()
    return nc


def _get_nc():
    if "nc" not in _NC_CACHE:
        _install_axon_ntff_hook()
        _NC_CACHE["nc"] = _build_nc()
    return _NC_CACHE["nc"]


def _host_prep(x, x_key, x_value, W_qk, w_shift):
    bf = ml_dtypes.bfloat16
    x = np.ascontiguousarray(np.asarray(x, dtype=np.float32))
    x_key = np.ascontiguousarray(np.asarray(x_key, dtype=np.float32))
    x_value = np.ascontiguousarray(np.asarray(x_value, dtype=np.float32))
    W_qk = np.ascontiguousarray(np.asarray(W_qk, dtype=np.float32))
    w_shift = np.asarray(w_shift, dtype=np.float32)

    xkT = np.ascontiguousarray(x_key.T)                      # [Q, T]
    wT = np.ascontiguousarray(W_qk.T)                        # [Q, H]; wT[f,h]=W[h,f]

    vcomb = np.zeros((T, NVC), np.float32)
    for s in range(NS):
        d = STEP * s
        vcomb[: T - d, NV * s : NV * (s + 1)] = x_value[d:, :]
    vcomb[:, NS * NV] = 1.0
    # pre-tile: [T, NVC] -> [NJ, P, NVC] -> [P, NJ*NVC] so each SBUF partition
    # line is one contiguous DMA descriptor
    vcomb = np.ascontiguousarray(
        vcomb.astype(bf).reshape(NJ, P, NVC).transpose(1, 0, 2).reshape(P, NJ * NVC)
    )

    # combine matrices (see aux layout comment in _build_nc)
    aux = np.zeros((P, 8 * P + NS * TAIL), np.float32)
    for s in range(NS):
        w = w_shift[0, s]
        for p in range(P):
            k = p - STEP * s
            if 0 <= k < P:
                aux[k, P * s + p] = w                      # A1
            kk = P + p - STEP * s
            if 0 <= kk < P:
                aux[kk, 4 * P + P * s + p] = w             # A2 (prev chunk)
        if s >= 1:
            for tp in range(TAIL):
                k = P + tp - STEP * s
                if 0 <= k < P:
                    aux[k, 8 * P + TAIL * s + tp] = w      # A3 (tail rows)
    aux = aux.astype(bf)

    in_maps = []
    for d in range(NCORES):
        r0 = d * M
        xT = np.ascontiguousarray(x[r0 : r0 + M].T)          # [Q, M]
        in_maps.append(
            {"xT": xT, "xkT": xkT, "wT": wT, "vcomb": vcomb, "aux": aux}
        )
    return in_maps


def kernel(x, x_key, x_value, W_qk, w_shift):
    global LAST_RESULT
    from concourse.bass_utils import run_bass_kernel_spmd

    nc = _get_nc()
    in_maps = _host_prep(x, x_key, x_value, W_qk, w_shift)
    res = run_bass_kernel_spmd(nc, in_maps, core_ids=list(range(NCORES)))
    LAST_RESULT = res
    out = np.concatenate(
        [res.results[d]["out"].reshape(M, NV) for d in range(NCORES)], axis=0
    )
    # add the 6-row cross-core overlap contributions
    for d in range(NCORES - 1):
        out[M * (d + 1) : M * (d + 1) + TAIL] += res.results[d]["outt"]
    return out.astype(np.float32)


# revision 25
# speedup vs baseline: 1.7100x; 1.0982x over previous
"""Trainium2 Bass kernel for shifted sparse attention (nn_Attention_74672301408506).

Math (reference):
    q = x @ W.T ; k = x_key @ W.T ; att = softmax(q k^T)
    out[i, v] = sum_s w_s * sum_j att[i-2s, j] * x_value[j+2s, v]

Device algorithm (8 cores, query rows sharded, 512 rows per core, no halo):
    per core, query rows [r0, r0+512):
      qT[h, i], kT[h, j]     (float32r = tf32 matmuls, full-rate at N>=256)
      S^T[j, i] = kT^T q     (scores transposed: keys on partitions)
      E = exp(S - 100)       (bf16; softmax is shift-invariant)
      Ru[i, :] = E^T @ [V_0|V_1|V_2|V_3|ones]   (ones col = softmax denominator)
      R = Ru[:, :320] / Ru[:, 320]
      C[t, v] = sum_s w_s R[t - 2s, 80s + v]  for t in [0, 518)  (banded matmuls)
    C rows [0, 512) are complete except the first 6 rows, which miss the
    neighbor's contribution; rows [512, 518) are exactly that contribution for
    the next core. The host adds the 6-row overlaps when unsharding (exact).
"""

import os
import sys
import types

import numpy as np
import ml_dtypes

T = 4096
Q = 256
H = 256
NV = 80
NS = 4
STEP = 2
NCORES = 8
M = T // NCORES            # 512 rows per core
CSUB = 100.0               # global score shift before exp
P = 128
NJ = T // P                # 32 key tiles
NF = Q // P                # 2 feature tiles
NH = H // P                # 2 hidden tiles
NMAIN = M // P             # 4 i-chunks of 128
NVC = NS * NV + 1          # value width incl ones column
TAIL = (NS - 1) * STEP     # 6 overlap rows between neighboring cores


def _install_axon_ntff_hook():
    """bass_utils' trace path imports antenv.axon_hooks, which the agent image
    lacks; shim it and register the ctypes-based NTFF profiler hook."""
    if "antenv.axon_hooks" in sys.modules:
        return
    try:
        import antenv
    except ImportError:
        return
    mod = types.ModuleType("antenv.axon_hooks")
    mod._hook = None
    mod.set_axon_ntff_profile_hook = lambda h: setattr(mod, "_hook", h)
    mod.get_axon_ntff_profile_hook = lambda: mod._hook
    sys.modules["antenv.axon_hooks"] = mod
    antenv.axon_hooks = mod
    try:
        from trn_agent_boot import trn_boot

        so_path = "/opt/axon/libaxon_pjrt.so"
        if os.path.exists(so_path):
            mod.set_axon_ntff_profile_hook(trn_boot._ntff_profile_via_ctypes(so_path))
    except Exception:
        pass


_NC_CACHE = {}
LAST_RESULT = None


def _build_nc():
    import concourse.mybir as mybir
    import concourse.tile as tile
    from concourse import bacc

    f32 = mybir.dt.float32
    f32r = mybir.dt.float32r
    bf16 = mybir.dt.bfloat16
    Exp = mybir.ActivationFunctionType.Exp

    nc = bacc.Bacc(None, target_bir_lowering=False)

    xT_d = nc.dram_tensor("xT", [Q, M], f32r, kind="ExternalInput")
    xkT_d = nc.dram_tensor("xkT", [Q, T], f32r, kind="ExternalInput")
    # G = W^T W (host-side): scores = x_key G x^T, so kT is never materialized
    g_d = nc.dram_tensor("gmat", [Q, Q], f32r, kind="ExternalInput")
    # pre-tiled on host: row p holds tile-row p of every j-tile (big DMA descs)
    vc_d = nc.dram_tensor("vcomb", [P, NJ * NVC], bf16, kind="ExternalInput")
    aux_d = nc.dram_tensor("aux", [P, 8 * P + NS * TAIL], bf16, kind="ExternalInput")
    out_d = nc.dram_tensor("out", [NMAIN, P, NV], f32, kind="ExternalOutput")
    outt_d = nc.dram_tensor("outt", [TAIL, NV], f32, kind="ExternalOutput")

    with tile.TileContext(nc) as tc:
        with (
            tc.tile_pool(name="consts", bufs=1) as consts,
            tc.tile_pool(name="io", bufs=1) as io,
            tc.tile_pool(name="store", bufs=1) as store,
            tc.tile_pool(name="small", bufs=6) as small,
            tc.tile_pool(name="psA", bufs=3, space="PSUM") as psA,
            tc.tile_pool(name="psR", bufs=4, space="PSUM") as psR,
            tc.tile_pool(name="psW", bufs=1, space="PSUM") as psW,
        ):
            # ---- PE warmup: dummy matmuls while input DMAs stream, so HAM
            # reaches K=8/8 (2.4 GHz) before real work ----
            wu = consts.tile([P, 512], bf16, name="wu")
            nc.vector.memset(wu, 0.0)
            wups = psW.tile([P, 512], f32, name="wups", tag="wu")
            for i in range(14):
                nc.tensor.matmul(wups, wu[:, 0:P], wu, start=True, stop=True)

            # ---- inputs; spread across HWDGE rings (sync+scalar) and SWDGE ----
            aux = consts.tile([P, 8 * P + NS * TAIL], bf16, name="aux")
            nc.gpsimd.dma_start(out=aux, in_=aux_d[:, :])
            gt = []
            for f in range(NF):
                t = consts.tile([P, Q], f32r, name=f"gt{f}", tag=f"gt{f}")
                eng = nc.sync if f == 0 else nc.scalar
                eng.dma_start(out=t, in_=g_d[P * f : P * (f + 1), :])
                gt.append(t)
            xt = []
            for f in range(NF):
                t = consts.tile([P, M], f32r, name=f"xt{f}", tag=f"xt{f}")
                eng = nc.sync if f == 0 else nc.scalar
                eng.dma_start(out=t, in_=xT_d[P * f : P * (f + 1), :])
                xt.append(t)
            xkh = []
            for f in range(NF):
                quarters = []
                for qq in range(4):
                    t = io.tile(
                        [P, T // 4], f32r, name=f"xkh{f}_{qq}", tag=f"xk{f}_{qq}"
                    )
                    eng = nc.sync if f == 0 else nc.scalar
                    eng.dma_start(
                        out=t,
                        in_=xkT_d[
                            P * f : P * (f + 1), (T // 4) * qq : (T // 4) * (qq + 1)
                        ],
                    )
                    quarters.append(t)
                xkh.append(quarters)
            # vcomb in 4 group-DMAs: group 0 early on the gpsimd ring; groups
            # 1-3 ride the sync/scalar HWDGE rings BEHIND the xk quarters (ring
            # FIFO order keeps them off the critical early HBM window)
            vcall = store.tile([P, NJ * NVC], bf16, name="vcall", tag="vc")
            GW = 8 * NVC
            for g, eng in [(0, nc.gpsimd), (1, nc.sync), (2, nc.scalar), (3, nc.sync)]:
                eng.dma_start(
                    out=vcall[:, GW * g : GW * (g + 1)],
                    in_=vc_d[:, GW * g : GW * (g + 1)],
                )
            vc = [vcall[:, NVC * j : NVC * (j + 1)] for j in range(NJ)]
            bias_t = consts.tile([P, 1], f32, name="bias_t")
            nc.vector.memset(bias_t, -CSUB)

            # ---- zT = G @ x^T  (the q-side projection through G) ----
            zt = []
            for h in range(NH):
                ps = psA.tile([P, M], f32, name=f"zps{h}", tag="ps")
                for f in range(NF):
                    nc.tensor.matmul(
                        ps,
                        gt[f][:, P * h : P * (h + 1)],
                        xt[f],
                        start=(f == 0),
                        stop=(f == NF - 1),
                    )
                t = store.tile([P, M], f32r, name=f"zt{h}", tag=f"zt{h}")
                if h == 0:
                    nc.vector.tensor_copy(t, ps)
                else:
                    nc.scalar.copy(t, ps)
                zt.append(t)

            # ---- pipeline: S^T(j) = x_keyT-tile^T @ zT directly from the DMA
            # quarters; exp; Ru lagging DELAY j-tiles ----
            ru = []
            for c in range(NMAIN):
                ru.append(psR.tile([P, NVC], f32, name=f"ru{c}", tag="ru"))
            elist = []
            DELAY = 2

            def ru_step(j):
                for c in range(NMAIN):
                    nc.tensor.matmul(
                        ru[c],
                        elist[j][:, P * c : P * (c + 1)],
                        vc[j],
                        start=(j == 0),
                        stop=(j == NJ - 1),
                    )

            for j in range(NJ):
                qq, jq = divmod(j, 8)
                ps = psA.tile([P, M], f32, name=f"sps{j}", tag="ps")
                for f in range(NF):
                    nc.tensor.matmul(
                        ps,
                        xkh[f][qq][:, P * jq : P * (jq + 1)],
                        zt[f],
                        start=(f == 0),
                        stop=(f == NF - 1),
                    )
                ej = store.tile([P, M], bf16, name=f"e{j}", tag="E", bufs=NJ)
                nc.scalar.activation(ej, ps, Exp, bias=bias_t)
                elist.append(ej)
                if j >= DELAY:
                    ru_step(j - DELAY)
            for j in range(NJ - DELAY, NJ):
                ru_step(j)

            # ---- normalize + combine, chunk by chunk ----
            # aux layout (bf16), all [128, 128] banded matrices with w_s baked:
            #   A1 = aux[:, 128s:128(s+1)]        k == p - 2s       (own chunk)
            #   A2 = aux[:, 512+128s:512+128(s+1)] k == 128 + p - 2s (prev chunk)
            #   A3 = aux[:, 1024+6s:1024+6(s+1)]  k == 128 + t' - 2s (tail rows)
            rch = []
            for c in range(NMAIN):
                rec = small.tile([P, 1], f32, name=f"rec{c}", tag="rec")
                nc.vector.reciprocal(rec, ru[c][:, NS * NV : NVC])
                t = store.tile([P, NS * NV], bf16, name=f"r{c}", tag=f"r{c}")
                nc.vector.tensor_scalar_mul(t, ru[c][:, 0 : NS * NV], rec)
                rch.append(t)

            oall = small.tile([P, NMAIN, NV], f32, name="oall", tag="osb")
            for c in range(NMAIN):
                po = psA.tile([P, NV], f32, name=f"po{c}", tag="ps")
                nmm = NS + (NS - 1 if c > 0 else 0)
                i = 0
                for s in range(NS):
                    nc.tensor.matmul(
                        po,
                        aux[:, P * s : P * (s + 1)],
                        rch[c][:, NV * s : NV * (s + 1)],
                        start=(i == 0),
                        stop=(i == nmm - 1),
                    )
                    i += 1
                if c > 0:
                    for s in range(1, NS):
                        nc.tensor.matmul(
                            po,
                            aux[:, 4 * P + P * s : 4 * P + P * (s + 1)],
                            rch[c - 1][:, NV * s : NV * (s + 1)],
                            start=False,
                            stop=(i == nmm - 1),
                        )
                        i += 1
                nc.vector.tensor_copy(oall[:, c, :], po)
            nc.sync.dma_start(out=out_d.rearrange("c p v -> p c v"), in_=oall)

            # tail rows [512, 518): next core's missing contribution
            pot = psA.tile([TAIL, NV], f32, name="pot", tag="ps")
            for s in range(1, NS):
                nc.tensor.matmul(
                    pot,
                    aux[:, 8 * P + TAIL * s : 8 * P + TAIL * (s + 1)],
                    rch[NMAIN - 1][:, NV * s : NV * (s + 1)],
                    start=(s == 1),
                    stop=(s == NS - 1),
                )
            ot = small.tile([TAIL, NV], f32, name="ot", tag="ot")
            nc.vector.tensor_copy(ot, pot)
            nc.sync.dma_start(out=outt_d[:, :], in_=ot)

    nc.compile()
    return nc


def _get_nc():
    if "nc" not in _NC_CACHE:
        _install_axon_ntff_hook()
        _NC_CACHE["nc"] = _build_nc()
    return _NC_CACHE["nc"]


def _host_prep(x, x_key, x_value, W_qk, w_shift):
    bf = ml_dtypes.bfloat16
    x = np.ascontiguousarray(np.asarray(x, dtype=np.float32))
    x_key = np.ascontiguousarray(np.asarray(x_key, dtype=np.float32))
    x_value = np.ascontiguousarray(np.asarray(x_value, dtype=np.float32))
    W_qk = np.ascontiguousarray(np.asarray(W_qk, dtype=np.float32))
    w_shift = np.asarray(w_shift, dtype=np.float32)

    xkT = np.ascontiguousarray(x_key.T)                      # [Q, T]
    gmat = np.ascontiguousarray(
        (W_qk.astype(np.float64).T @ W_qk.astype(np.float64)).astype(np.float32)
    )                                                        # [Q, Q], symmetric

    vcomb = np.zeros((T, NVC), np.float32)
    for s in range(NS):
        d = STEP * s
        vcomb[: T - d, NV * s : NV * (s + 1)] = x_value[d:, :]
    vcomb[:, NS * NV] = 1.0
    # pre-tile: [T, NVC] -> [NJ, P, NVC] -> [P, NJ*NVC] so each SBUF partition
    # line is one contiguous DMA descriptor
    vcomb = np.ascontiguousarray(
        vcomb.astype(bf).reshape(NJ, P, NVC).transpose(1, 0, 2).reshape(P, NJ * NVC)
    )

    # combine matrices (see aux layout comment in _build_nc)
    aux = np.zeros((P, 8 * P + NS * TAIL), np.float32)
    for s in range(NS):
        w = w_shift[0, s]
        for p in range(P):
            k = p - STEP * s
            if 0 <= k < P:
                aux[k, P * s + p] = w                      # A1
            kk = P + p - STEP * s
            if 0 <= kk < P:
                aux[kk, 4 * P + P * s + p] = w             # A2 (prev chunk)
        if s >= 1:
            for tp in range(TAIL):
                k = P + tp - STEP * s
                if 0 <= k < P:
                    aux[k, 8 * P + TAIL * s + tp] = w      # A3 (tail rows)
    aux = aux.astype(bf)

    in_maps = []
    for d in range(NCORES):
        r0 = d * M
        xT = np.ascontiguousarray(x[r0 : r0 + M].T)          # [Q, M]
        in_maps.append(
            {"xT": xT, "xkT": xkT, "gmat": gmat, "vcomb": vcomb, "aux": aux}
        )
    return in_maps


def kernel(x, x_key, x_value, W_qk, w_shift):
    global LAST_RESULT
    from concourse.bass_utils import run_bass_kernel_spmd

    nc = _get_nc()
    in_maps = _host_prep(x, x_key, x_value, W_qk, w_shift)
    res = run_bass_kernel_spmd(nc, in_maps, core_ids=list(range(NCORES)))
    LAST_RESULT = res
    out = np.concatenate(
        [res.results[d]["out"].reshape(M, NV) for d in range(NCORES)], axis=0
    )
    # add the 6-row cross-core overlap contributions
    for d in range(NCORES - 1):
        out[M * (d + 1) : M * (d + 1) + TAIL] += res.results[d]["outt"]
    return out.astype(np.float32)
